# revision 6
# baseline (speedup 1.0000x reference)
"""Trainium2 Bass kernel for nn_ModalityMoERouter (expert-choice MoE routing).

Contract: kernel(**inputs) takes the FULL inputs from reference.setup_inputs()
and returns (dispatch, combine), each (16, 8192, 16) float32.

Sharding: data-parallel over batch B=16 across 8 NeuronCores (2 batches/core);
gate weights and expert centers replicated. The global mean(dists) scalar is
computed with one AllReduce (overlapped with the gate MLP).

Math notes:
 - The hard-cap + redistribution step in the reference is exactly a no-op:
   dispatch after the routing floor is <= 0.4*sigmoid + 0.0375 <= 0.4375,
   while cap >= 0.5, so excess == 0 bitwise. It is therefore skipped (t unused).
 - Expert-choice top-k (k=1024 of N=8192 per (batch, expert)) is realized as
   a threshold: branch-free bisection on the count of logits above a candidate
   converges to theta; mask = logits > theta.
 - xyz's contribution to the gate MLP is folded into the tokens on the host:
   tokens' = tokens + xyz @ Q with Q = W1[512:515] @ pinv(W1[:512]) (exact in
   real arithmetic since W1[:512] has full column rank), so the device GEMM is
   a clean 512->256 contraction with no PE transposes (tokens arrive
   host-transposed as [D, N]).
 - b2 is folded into the final Sigmoid's bias; the bisection threshold then
   applies to b2-less logits, which selects the same tokens.
 - Distances are one matmul per (b, blk): dist^2 = |x|^2 - 2 x.c + |c|^2 via a
   block-diagonal [40, 128] stationary (8 token-groups x [x,y,z,|x|^2,1]).
 - All big matmuls run as float32r (1 cycle/row when free >= 256, vs 4 for
   fp32), keeping fp32 storage and ~1e-5 relative GEMM precision.
 - W2 is applied with g-masked [128, 128] stationaries so all 8 token-groups
   of a (b, blk) accumulate directly into one [128, 512] psum in the final
   logits layout - no per-tile PSUM->SBUF copy or SBUF->SBUF DMA.
 - emit(0)'s work is issued interleaved into bisect(1)'s iterations so the
   in-order vector queue fills the bisection's cross-engine wait gaps.

On-chip layout (per core): flat tiles [128, 2048] with partition p = g*16+e
and free f = b*1024 + blk*512 + t, where token n = (blk*8+g)*512 + t.
"""

import numpy as np

B = 16
N = 8192
D = 512
H = 256
E = 16
N_CORES = 8
BPC = B // N_CORES
NT = N // 512               # 16 tiles of 512 tokens per batch
KSEL = N * 2 // E           # 1024
ALPHA = min(min(0.05, 0.15 / 4) * E, 1.0)
DSCALE = 1.0 - ALPHA        # 0.4
DFLOOR = ALPHA / E          # 0.0375
N_ITER = 20
W0 = 16.0                   # bisection range [LO0, LO0 + W0]
LO0 = -10.0
_DEBUG = False

_prog_cache = {}


def _build(debug=False):
    import concourse.bacc as bacc
    import concourse.mybir as mybir
    import concourse.tile as tile

    F32 = mybir.dt.float32
    F32R = mybir.dt.float32r
    AO = mybir.AluOpType
    AF = mybir.ActivationFunctionType
    AX = mybir.AxisListType

    nc = bacc.Bacc("TRN2", num_devices=N_CORES)

    tokT_d = nc.dram_tensor("tokT", [BPC, D, N], F32R, kind="ExternalInput")
    xg40_d = nc.dram_tensor("xg40", [40, BPC * 1024], F32R, kind="ExternalInput")
    cstat_d = nc.dram_tensor("cstat", [40, 128], F32R, kind="ExternalInput")
    w1_d = nc.dram_tensor("W1", [D, H], F32R, kind="ExternalInput")
    w2g_d = nc.dram_tensor("w2g", [2, 8, 128, 128], F32R, kind="ExternalInput")
    b1_d = nc.dram_tensor("b1", [H], F32, kind="ExternalInput")
    b2bc_d = nc.dram_tensor("b2bc", [128, 1], F32, kind="ExternalInput")
    ident_d = nc.dram_tensor("ident", [128, 128], F32, kind="ExternalInput")
    m2_d = nc.dram_tensor("m2", [128, 128], F32, kind="ExternalInput")

    disp_d = nc.dram_tensor("disp", [BPC, N, E], F32, kind="ExternalOutput")
    comb_d = nc.dram_tensor("comb", [BPC, N, E], F32, kind="ExternalOutput")
    if debug:
        dbg_logits_d = nc.dram_tensor("dbg_logits", [128, 2048], F32,
                                      kind="ExternalOutput")
        dbg_dists_d = nc.dram_tensor("dbg_dists", [128, 2048], F32,
                                     kind="ExternalOutput")

    with tile.TileContext(nc) as tc:
        with tc.tile_pool(name="const", bufs=1) as cpool, \
             tc.tile_pool(name="big", bufs=1) as bigpool, \
             tc.tile_pool(name="work", bufs=2) as work, \
             tc.tile_pool(name="ps", bufs=2, space="PSUM") as ps, \
             tc.tile_pool(name="dram", bufs=1, space="DRAM") as dram:

            # ---- constants ----
            w1_sb = []
            for kc in range(4):
                row = []
                for mc in range(2):
                    t = cpool.tile([128, 128], F32R, tag=f"w1_{kc}_{mc}",
                                   name=f"w1_{kc}_{mc}")
                    nc.sync.dma_start(
                        out=t[:], in_=w1_d[kc * 128:(kc + 1) * 128,
                                           mc * 128:(mc + 1) * 128])
                    row.append(t)
                w1_sb.append(row)
            w2g_sb = []
            for mc in range(2):
                row = []
                for g in range(8):
                    t = cpool.tile([128, 128], F32R, tag=f"w2g_{mc}_{g}",
                                   name=f"w2g_{mc}_{g}")
                    nc.sync.dma_start(out=t[:], in_=w2g_d[mc, g])
                    row.append(t)
                w2g_sb.append(row)
            b1_sb = []
            for mc in range(2):
                t = cpool.tile([128, 1], F32, tag=f"b1_{mc}", name=f"b1_{mc}")
                nc.sync.dma_start(out=t[:],
                                  in_=b1_d[mc * 128:(mc + 1) * 128].unsqueeze(1))
                b1_sb.append(t)
            b2bc_sb = cpool.tile([128, 1], F32, tag="b2bc", name="b2bc")
            nc.sync.dma_start(out=b2bc_sb[:], in_=b2bc_d[:])
            cstat_sb = cpool.tile([40, 128], F32R, tag="cstat", name="cstat")
            nc.sync.dma_start(out=cstat_sb[:], in_=cstat_d[:])
            xg40_sb = cpool.tile([40, BPC * 1024], F32R, tag="xg40", name="xg40")
            nc.sync.dma_start(out=xg40_sb[:], in_=xg40_d[:])
            ident_sb = cpool.tile([128, 128], F32, tag="ident", name="ident")
            nc.sync.dma_start(out=ident_sb[:], in_=ident_d[:])
            m2_sb = cpool.tile([128, 128], F32, tag="m2", name="m2")
            nc.sync.dma_start(out=m2_sb[:], in_=m2_d[:])
            ones_1x128 = cpool.tile([1, 128], F32, tag="o1x", name="o1x")
            nc.vector.memset(ones_1x128[:], 1.0)
            ones_128x1 = cpool.tile([128, 1], F32, tag="ox1", name="ox1")
            nc.vector.memset(ones_128x1[:], 1.0)
            ones_wide = cpool.tile([128, 1024], F32, tag="onesw", name="onesw")
            nc.vector.memset(ones_wide[:], 1.0)

            # ---- persistent tiles ----
            logits_A = bigpool.tile([128, 2048], F32, tag="logits", name="logits")
            dists_A = bigpool.tile([128, 2048], F32, tag="dists", name="dists")
            sig_A = bigpool.tile([128, 2048], F32, tag="sig", name="sig")

            # ============ Phase A: distances + global mean =================
            for b in range(BPC):
                for blk in range(2):
                    off = b * 1024 + blk * 512
                    p_d = ps.tile([128, 512], F32, tag="ph", name="p_d", bufs=3)
                    nc.tensor.matmul(p_d[:], cstat_sb[:],
                                     xg40_sb[:, off:off + 512],
                                     start=True, stop=True)
                    nc.scalar.activation(dists_A[:, off:off + 512], p_d[:],
                                         AF.Sqrt)

            rsum = work.tile([128, 1], F32, tag="rsum", name="rsum")
            nc.vector.tensor_reduce(out=rsum[:], in_=dists_A[:], axis=AX.X, op=AO.add)
            p_tot = ps.tile([1, 1], F32, tag="pcnt", name="p_tot", bufs=1)
            nc.tensor.matmul(p_tot[:], ones_128x1[:], rsum[:], start=True, stop=True)
            s_tot = work.tile([1, 1], F32, tag="stot", name="stot")
            nc.vector.tensor_copy(s_tot[:], p_tot[:])
            p_bc = ps.tile([128, 1], F32, tag="pcnt", name="p_bc", bufs=1)
            nc.tensor.matmul(p_bc[:], ones_1x128[:], s_tot[:], start=True, stop=True)
            sb_bc = work.tile([128, 1], F32, tag="sbbc", name="sbbc")
            nc.vector.tensor_copy(sb_bc[:], p_bc[:])
            cc_in = dram.tile([128, 1], F32)
            cc_out = dram.tile([128, 1], F32, addr_space="Shared")
            nc.sync.dma_start(out=cc_in[:], in_=sb_bc[:])
            nc.gpsimd.collective_compute(
                "AllReduce", AO.add, ins=[cc_in.opt()], outs=[cc_out.opt()],
                replica_groups=[list(range(N_CORES))])
            S_sb = bigpool.tile([128, 1], F32, tag="S", name="S")
            nc.sync.dma_start(out=S_sb[:], in_=cc_out[:])
            m_sb = bigpool.tile([128, 1], F32, tag="m", name="m")
            nc.vector.tensor_scalar(out=m_sb[:], in0=S_sb[:],
                                    scalar1=1.0 / (B * N * E), scalar2=1e-6,
                                    op0=AO.mult, op1=AO.add)
            r_sb = bigpool.tile([128, 1], F32, tag="r", name="r")
            nc.vector.reciprocal(r_sb[:], m_sb[:])
            a_sb = bigpool.tile([128, 1], F32, tag="a", name="a")
            nc.vector.tensor_scalar(out=a_sb[:], in0=r_sb[:], scalar1=-1.0,
                                    scalar2=None, op0=AO.mult)

            # ---- bisect state ----
            lo = []
            for b in range(BPC):
                lo.append(bigpool.tile([128, 1], F32, tag=f"lo{b}", name=f"lo{b}"))
                nc.vector.memset(lo[b][:], LO0)
            scr = []
            for b in range(BPC):
                scr.append(bigpool.tile([128, 1024], F32, tag=f"scr{b}",
                                        name=f"scr{b}"))

            def mlp_blk(b, blk):
                """MLP for one (b, blk): 8 token-group tiles accumulating
                their content logits into one [128, 512] psum via g-masked W2,
                then fused with the spatial affinity into logits_A."""
                off = b * 1024 + blk * 512
                p_L = ps.tile([128, 512], F32, tag="pL", name="p_L", bufs=2)
                for g in range(8):
                    T = blk * 8 + g
                    mvc = []
                    for kc in range(4):
                        t = work.tile([128, 512], F32R, tag="mvc", name="mvc",
                                      bufs=12)
                        nc.sync.dma_start(
                            out=t[:],
                            in_=tokT_d[b, kc * 128:(kc + 1) * 128,
                                       T * 512:(T + 1) * 512])
                        mvc.append(t)
                    for mc in range(2):
                        p_h = ps.tile([128, 512], F32, tag="ph", name="p_h",
                                      bufs=3)
                        for kc in range(4):
                            nc.tensor.matmul(p_h[:], w1_sb[kc][mc][:],
                                             mvc[kc][:],
                                             start=(kc == 0), stop=(kc == 3))
                        t_h = work.tile([128, 512], F32R, tag=f"h{mc}",
                                        name=f"h{mc}", bufs=3)
                        nc.scalar.activation(t_h[:], p_h[:], AF.Gelu,
                                             bias=b1_sb[mc][:], scale=1.0)
                        nc.tensor.matmul(p_L[:], w2g_sb[mc][g][:], t_h[:],
                                         start=(g == 0 and mc == 0),
                                         stop=(g == 7 and mc == 1),
                                         skip_group_check=True)
                # logits (without b2) = W2-content + a*dists
                nc.vector.scalar_tensor_tensor(
                    out=logits_A[:, off:off + 512], in0=dists_A[:, off:off + 512],
                    scalar=a_sb[:], in1=p_L[:], op0=AO.mult, op1=AO.add)

            def bisect(b, fillers=()):
                """20 bisection iterations for batch b's expert thresholds.
                After each iteration one pending filler closure is issued so
                independent work interleaves into the chain's wait gaps."""
                fillers = list(fillers)
                sl = slice(b * 1024, (b + 1) * 1024)
                t_mid = work.tile([128, 1], F32, tag=f"mid{b}", name=f"mid{b}",
                                  bufs=2)
                t_acc = work.tile([128, 1], F32, tag=f"pacc{b}", name=f"pacc{b}",
                                  bufs=2)
                t_s = work.tile([128, 1], F32, tag=f"sel{b}", name=f"sel{b}",
                                bufs=2)
                for i in range(N_ITER):
                    w = W0 / (2 ** (i + 1))
                    nc.vector.tensor_scalar(out=t_mid[:], in0=lo[b][:],
                                            scalar1=w, scalar2=None, op0=AO.add)
                    nc.vector.scalar_tensor_tensor(
                        out=scr[b][:], in0=logits_A[:, sl], scalar=t_mid[:],
                        in1=ones_wide[:], op0=AO.is_gt, op1=AO.mult,
                        accum_out=t_acc[:])
                    p_cnt = ps.tile([128, 1], F32, tag="pcnt", name="p_cnt",
                                    bufs=1)
                    nc.tensor.matmul(p_cnt[:], m2_sb[:], t_acc[:],
                                     start=True, stop=True)
                    nc.vector.tensor_scalar(out=t_s[:], in0=p_cnt[:],
                                            scalar1=float(KSEL), scalar2=None,
                                            op0=AO.is_ge)
                    nc.vector.scalar_tensor_tensor(
                        out=lo[b][:], in0=t_s[:], scalar=w, in1=lo[b][:],
                        op0=AO.mult, op1=AO.add)
                    if fillers:
                        fillers.pop(0)()
                for f in fillers:
                    f()

            def emit_chunks(b):
                """Return emit work for batch b as a list of closures."""
                sl = slice(b * 1024, (b + 1) * 1024)

                def preamble():
                    # sigmoid with b2 folded in as the activation bias
                    nc.scalar.activation(sig_A[:, sl], logits_A[:, sl],
                                         AF.Sigmoid, bias=b2bc_sb[:], scale=1.0)

                def gate():
                    nc.vector.scalar_tensor_tensor(
                        out=logits_A[:, sl], in0=logits_A[:, sl],
                        scalar=lo[b][:], in1=sig_A[:, sl],
                        op0=AO.is_gt, op1=AO.mult)
                    nc.vector.tensor_scalar(out=logits_A[:, sl],
                                            in0=logits_A[:, sl],
                                            scalar1=DSCALE, scalar2=DFLOOR,
                                            op0=AO.mult, op1=AO.add)

                out_view_d = disp_d[b].rearrange(
                    "(blk g q t) e -> blk q t g e", blk=2, g=8, q=4)
                out_view_c = comb_d[b].rearrange(
                    "(blk g q t) e -> blk q t g e", blk=2, g=8, q=4)

                def chunk(blk, q):
                    def run():
                        off = b * 1024 + blk * 512 + q * 128
                        p_o = ps.tile([128, 128], F32, tag="ph", name="p_o",
                                      bufs=3)
                        nc.tensor.transpose(p_o[:], logits_A[:, off:off + 128],
                                            ident_sb[:])
                        t_o = work.tile([128, 128], F32, tag="outT",
                                        name="outT", bufs=3)
                        nc.vector.tensor_copy(t_o[:], p_o[:])
                        t_sden = work.tile([128, 8], F32, tag="sden",
                                           name="sden", bufs=3)
                        nc.vector.tensor_reduce(
                            out=t_sden[:],
                            in_=t_o[:].rearrange("t (g e) -> t g e", g=8),
                            axis=AX.X, op=AO.add)
                        nc.vector.tensor_scalar(out=t_sden[:], in0=t_sden[:],
                                                scalar1=1e-8, scalar2=None,
                                                op0=AO.add)
                        t_rden = work.tile([128, 8], F32, tag="rden",
                                           name="rden", bufs=3)
                        nc.vector.reciprocal(t_rden[:], t_sden[:])
                        t_c = work.tile([128, 128], F32, tag="outC",
                                        name="outC", bufs=3)
                        nc.vector.tensor_tensor(
                            out=t_c[:].rearrange("t (g e) -> t g e", g=8),
                            in0=t_o[:].rearrange("t (g e) -> t g e", g=8),
                            in1=t_rden[:].unsqueeze(2).broadcast_to([128, 8, E]),
                            op=AO.mult)
                        nc.sync.dma_start(
                            out=out_view_d[blk, q],
                            in_=t_o[:].rearrange("t (g e) -> t g e", g=8))
                        nc.sync.dma_start(
                            out=out_view_c[blk, q],
                            in_=t_c[:].rearrange("t (g e) -> t g e", g=8))
                    return run

                return ([preamble, gate]
                        + [chunk(blk, q) for blk in range(2) for q in range(4)])

            # ---- schedule ----
            mlp_blk(0, 0)
            mlp_blk(0, 1)
            bisect(0)                        # overlaps MLP(1) on the tensor side
            mlp_blk(1, 0)
            mlp_blk(1, 1)
            if debug:
                nc.sync.dma_start(out=dbg_dists_d[:], in_=dists_A[:])
                nc.sync.dma_start(out=dbg_logits_d[:], in_=logits_A[:])
            e0 = emit_chunks(0)
            e1 = emit_chunks(1)
            # sig(1) only needs logits(1): issue it as a filler too
            bisect(1, fillers=e0 + [e1[0]])
            for f in e1[1:]:
                f()

    nc.finalize()
    return nc


def _get_prog(debug=False):
    key = ("prog", debug)
    if key not in _prog_cache:
        _prog_cache[key] = _build(debug)
    return _prog_cache[key]


def make_in_maps(inputs):
    tokens = np.asarray(inputs["tokens"], dtype=np.float32)
    xyz = np.asarray(inputs["spatial_xyz"], dtype=np.float32)
    W1 = np.asarray(inputs["W1"], dtype=np.float32)
    b1 = np.asarray(inputs["b1"], dtype=np.float32)
    W2 = np.asarray(inputs["W2"], dtype=np.float32)
    b2 = np.asarray(inputs["b2"], dtype=np.float32)
    centers = np.asarray(inputs["centers"], dtype=np.float32)

    # fold xyz @ W1[512:515] into the tokens: Q = W1b @ pinv(W1a)
    W1a = W1[:D].astype(np.float64)
    W1b = W1[D:].astype(np.float64)
    Q = W1b @ np.linalg.pinv(W1a)                            # (3, 512)
    tokp = tokens.astype(np.float64) + xyz.astype(np.float64) @ Q
    tokT = np.ascontiguousarray(
        tokp.transpose(0, 2, 1)).astype(np.float32)          # (B, D, N)

    b2bc = np.ascontiguousarray(np.tile(b2, 8)[:, None].astype(np.float32))
    ident = np.eye(128, dtype=np.float32)
    m2 = np.ascontiguousarray(
        (np.arange(128)[:, None] % 16 == np.arange(128)[None, :] % 16)
        .astype(np.float32))

    # block-diagonal distance stationary: rows 5g+c, cols p=g*16+e
    cvec = np.zeros((5, E), np.float32)
    cvec[0:3] = -2.0 * centers.T
    cvec[3] = 1.0
    cvec[4] = (centers * centers).sum(-1)
    cstat = np.zeros((40, 128), np.float32)
    for g in range(8):
        cstat[5 * g:5 * g + 5, 16 * g:16 * g + 16] = cvec

    # g-masked W2 stationaries: w2g[mc, g, k, p] = W2[mc*128+k, e] iff g(p)==g
    w2g = np.zeros((2, 8, 128, 128), np.float32)
    for mc in range(2):
        for g in range(8):
            w2g[mc, g, :, 16 * g:16 * g + 16] = W2[mc * 128:(mc + 1) * 128, :]

    in_maps = []
    for c in range(N_CORES):
        sl = slice(BPC * c, BPC * (c + 1))
        xl = xyz[sl]                                          # (BPC, N, 3)
        # xg40 rows 5g+c: [x, y, z, |x|^2, 1] for tokens (blk*8+g)*512 + t
        xg = np.empty((40, BPC * 1024), np.float32)
        for b in range(BPC):
            xs = xl[b].reshape(NT, 512, 3)                    # (T, t, c)
            for blk in range(2):
                for g in range(8):
                    seg = xs[blk * 8 + g]                     # (512, 3)
                    col = slice(b * 1024 + blk * 512,
                                b * 1024 + blk * 512 + 512)
                    xg[5 * g:5 * g + 3, col] = seg.T
                    xg[5 * g + 3, col] = (seg * seg).sum(-1)
                    xg[5 * g + 4, col] = 1.0
        in_maps.append({
            "tokT": tokT[sl],
            "xg40": np.ascontiguousarray(xg),
            "cstat": cstat,
            "W1": np.ascontiguousarray(W1[:D]),
            "w2g": w2g,
            "b1": b1,
            "b2bc": b2bc,
            "ident": ident,
            "m2": m2,
        })
    return in_maps


def kernel(**inputs):
    from concourse.bass_utils import run_bass_kernel_spmd

    nc = _get_prog(_DEBUG)
    in_maps = make_in_maps(inputs)
    res = run_bass_kernel_spmd(nc, in_maps, list(range(N_CORES)))
    dispatch = np.concatenate([res.results[c]["disp"] for c in range(N_CORES)], axis=0)
    combine = np.concatenate([res.results[c]["comb"] for c in range(N_CORES)], axis=0)
    if _DEBUG:
        kernel._dbg = [(res.results[c]["dbg_logits"], res.results[c]["dbg_dists"])
                       for c in range(N_CORES)]
    return dispatch, combine
